# revision 26
# baseline (speedup 1.0000x reference)
"""2-layer GCN (DGCN) on 8 TRN2 NeuronCores — v3.

Strategy (graph/data parallel, dst-sharded):
  - Segment-sum via one-hot matmuls accumulated in PSUM. One-hot matrices
    are streamed from HBM as fp8 (exact 0/1) and fed to the PE directly
    (mixed fp8 x bf16 matmul) — half of v1's one-hot bytes, no DVE cost.
  - Per-(tile,half) chunk counts padded only to the max over the 8 cores
    (uniform SPMD program); self-loops dropped from the L2 gather (the DVE
    adds the local y2 rows into PSUM instead).
  - The y2 AllGather is split into 2 piece-collectives; the first overlaps
    the second half of layer-1 compute.
  - Layer-2 row gathers are split into prepare_only descriptor generation
    (SWDGE, starts at t=0 on the 4 Q7 queue pairs, overlapping layer 1 and
    the collectives) and trigger_dma at consumption time.

Layer math (PyG GCNConv, S = D^-1/2 (A+I) D^-1/2):
  L1: psu[f,d] += xg_chunk[slot,f]^T oh[slot,d]  (xg = dis_src*x_src host-
      pregathered incl self-loops); ps2 = psu^T@W1 + invdis(x)b1;
      res = Relu(dis*ps2); h = res + x; y2 = dis*(h@W2) bf16 per tile.
  AllGather y2 (2 pieces) -> full y2 table in DRAM.
  L2: ps[d,f] = invdis(x)b2 + sum_ch oh^T gb_chunk + y2own; out = dis*ps.
"""

import hashlib
import numpy as np
import ml_dtypes

import concourse.bass as bass
import concourse.bacc as bacc
import concourse.tile as tile
import concourse.mybir as mybir
from concourse.bass_utils import run_bass_kernel_spmd

N_CORES = 8
N_REAL = 50000
N_PAD = 50176
SHARD = N_PAD // N_CORES       # 6272
TILES = SHARD // 128           # 49
FEAT = 128
GROUP = 3                      # dst tiles per group (last group = 1)
P0_TILES = 24                  # table piece 0 = tiles 0..23 of each shard
P1_TILES = TILES - P0_TILES    # 25
P0_ROWS = P0_TILES * 128       # 3072 per core
P1_ROWS = P1_TILES * 128       # 3200 per core

F32 = mybir.dt.float32
BF16 = mybir.dt.bfloat16
FP8 = mybir.dt.float8e4
NPBF = ml_dtypes.bfloat16
NPF8 = ml_dtypes.float8_e4m3

_GROUPS = [list(range(g, min(g + GROUP, TILES))) for g in range(0, TILES, GROUP)]
NG = len(_GROUPS)


def _piece_rows(n):
    """Global node id -> (piece, row within piece table)."""
    c = n // SHARD
    r = n % SHARD
    p = (r >= P0_ROWS).astype(np.int64)
    row = np.where(p == 0, c * P0_ROWS + r, c * P1_ROWS + (r - P0_ROWS))
    return p, row


def _onehot3(dloc, S, dt=NPF8):
    """[S] float dst-slot values (-512 pads) -> [128, S/128, 128]."""
    oh = np.zeros((S, 128), dt)
    valid = dloc >= 0
    oh[np.nonzero(valid)[0], dloc[valid].astype(np.int64)] = 1.0
    return oh.reshape(S // 128, 128, 128).transpose(1, 0, 2).copy()


def _preprocess(edge_index, x, W1, b1, W2, b2):
    src = np.asarray(edge_index[0], dtype=np.int64)
    dst = np.asarray(edge_index[1], dtype=np.int64)
    loops = np.arange(N_REAL, dtype=np.int64)
    src1 = np.concatenate([src, loops])
    dst1 = np.concatenate([dst, loops])

    deg = np.bincount(dst1, minlength=N_PAD).astype(np.float64)
    with np.errstate(divide="ignore"):
        dis = np.where(deg > 0, 1.0 / np.sqrt(deg), 0.0).astype(np.float32)
    invdis = np.where(deg > 0, np.sqrt(deg), 0.0).astype(np.float32)

    xp = np.zeros((N_PAD, FEAT), np.float32)
    xp[:N_REAL] = np.asarray(x, np.float32)
    disx = np.zeros((N_PAD + 1, FEAT), np.float32)   # +1 zero row for pads
    disx[:N_PAD] = dis[:, None] * xp

    core1 = dst1 // SHARD
    tile1 = (dst1 % SHARD) // 128
    core2 = dst // SHARD
    tile2 = (dst % SHARD) // 128
    half2, row2 = _piece_rows(src)

    cnt1 = np.zeros((N_CORES, TILES), np.int64)
    np.add.at(cnt1, (core1, tile1), 1)
    nch1 = np.maximum(1, np.ceil(cnt1.max(0) / 128).astype(np.int64))  # [49]

    cnt2 = np.zeros((N_CORES, TILES, 2), np.int64)
    np.add.at(cnt2, (core2, tile2, half2), 1)
    nch2 = np.maximum(1, np.ceil(cnt2.max(0) / 128).astype(np.int64))  # [49,2]

    spec = (tuple(nch1.tolist()), tuple(map(tuple, nch2.tolist())))

    S1 = int(nch1.sum()) * 128
    cb1 = np.zeros(TILES, np.int64)
    cb1[1:] = np.cumsum(nch1)[:-1]
    gid = np.repeat(np.arange(NG), [len(g) for g in _GROUPS])
    order_th = []
    for grp in _GROUPS:
        for h in (0, 1):
            for t in grp:
                order_th.append((t, h))
    cb2 = {}
    acc = 0
    for (t, h) in order_th:
        cb2[(t, h)] = acc
        acc += int(nch2[t, h])
    NCH2 = acc
    S2 = NCH2 * 128

    per_core = []
    for c in range(N_CORES):
        # ---- L1 ----
        m1 = core1 == c
        s1c, d1c, t1c = src1[m1], dst1[m1], tile1[m1]
        o = np.lexsort((s1c, d1c))
        s1c, d1c, t1c = s1c[o], d1c[o], t1c[o]
        slot1 = np.empty(len(s1c), np.int64)
        for t in range(TILES):
            m = t1c == t
            slot1[m] = cb1[t] * 128 + np.arange(int(m.sum()))
        src_map1 = np.full(S1, N_PAD, np.int64)
        src_map1[slot1] = s1c
        dloc1 = np.full(S1, -512.0, np.float32)
        dloc1[slot1] = (d1c & 127).astype(np.float32)

        xg = disx[src_map1].astype(NPF8)
        xg3 = xg.reshape(S1 // 128, 128, FEAT).transpose(1, 0, 2).copy()
        oh1 = _onehot3(dloc1, S1)

        # ---- L2 ----
        m2 = core2 == c
        s2c, d2c, t2c, h2c, r2c = src[m2], dst[m2], tile2[m2], half2[m2], row2[m2]
        o = np.lexsort((s2c, d2c, t2c, h2c, gid[t2c]))
        s2c, d2c, t2c, h2c, r2c = s2c[o], d2c[o], t2c[o], h2c[o], r2c[o]
        slot2 = np.empty(len(s2c), np.int64)
        for (t, h) in order_th:
            m = (t2c == t) & (h2c == h)
            slot2[m] = cb2[(t, h)] * 128 + np.arange(int(m.sum()))
        idx2 = np.zeros(S2, np.int64)                # pad -> row 0 (real)
        idx2[slot2] = r2c
        dloc2 = np.full(S2, -512.0, np.float32)
        dloc2[slot2] = (d2c & 127).astype(np.float32)
        oh2 = _onehot3(dloc2, S2, NPBF)

        idx16 = idx2.astype(np.int16).reshape(-1, 16).T.copy()
        idx128 = np.tile(idx16, (8, 1))

        sl = slice(c * SHARD, (c + 1) * SHARD)
        xs = xp[sl]
        x_sb = xs.reshape(TILES, 128, FEAT).transpose(1, 0, 2).reshape(128, SHARD)
        per_core.append({
            "xg": xg3,
            "oh1": oh1,
            "oh2": oh2,
            "idx": idx128,
            "x_sb": np.ascontiguousarray(x_sb),
            "dis": np.ascontiguousarray(dis[sl].reshape(TILES, 128).T),
            "invdis": invdis[sl][None, :].astype(NPBF),
            "W1": np.asarray(W1, np.float32).astype(NPBF),
            "W2": np.asarray(W2, np.float32).astype(NPBF),
            "b1": np.asarray(b1, np.float32)[None, :].astype(NPBF),
            "b2": np.asarray(b2, np.float32)[None, :].astype(NPBF),
            "ident": np.eye(128, dtype=np.float32),
        })

    return per_core, spec, nch1, nch2, cb1, cb2, S1, S2, NCH2


def _build(nch1, nch2, cb1, cb2, S2, NCH2, compile=True):
    nc = bacc.Bacc("TRN2", target_bir_lowering=False, debug=False,
                   num_devices=N_CORES, num_swdge_queues=4)

    NCH1 = int(nch1.sum())
    g_nch1 = [int(sum(nch1[t] for t in grp)) for grp in _GROUPS]
    g_nch2 = [int(sum(nch2[t][h] for t in grp for h in (0, 1)))
              for grp in _GROUPS]
    NCHMAX1 = max(g_nch1)
    NCHMAX2 = max(g_nch2)

    xg_d = nc.dram_tensor("xg", [128, NCH1, 128], FP8, kind="ExternalInput")
    oh1_d = nc.dram_tensor("oh1", [128, NCH1, 128], FP8, kind="ExternalInput")
    oh2_d = nc.dram_tensor("oh2", [128, NCH2, 128], BF16, kind="ExternalInput")
    idx_d = nc.dram_tensor("idx", [128, S2 // 16], mybir.dt.int16,
                           kind="ExternalInput")
    xsb_d = nc.dram_tensor("x_sb", [128, SHARD], F32, kind="ExternalInput")
    dis_d = nc.dram_tensor("dis", [128, TILES], F32, kind="ExternalInput")
    invdis_d = nc.dram_tensor("invdis", [1, SHARD], BF16, kind="ExternalInput")
    W1_d = nc.dram_tensor("W1", [128, 128], BF16, kind="ExternalInput")
    W2_d = nc.dram_tensor("W2", [128, 128], BF16, kind="ExternalInput")
    b1_d = nc.dram_tensor("b1", [1, 128], BF16, kind="ExternalInput")
    b2_d = nc.dram_tensor("b2", [1, 128], BF16, kind="ExternalInput")
    ident_d = nc.dram_tensor("ident", [128, 128], F32, kind="ExternalInput")
    out_d = nc.dram_tensor("out", [SHARD, FEAT], F32, kind="ExternalOutput")

    y2s = [nc.dram_tensor("y2s0", [P0_ROWS, FEAT], BF16, kind="Internal"),
           nc.dram_tensor("y2s1", [P1_ROWS, FEAT], BF16, kind="Internal")]
    y2f = [nc.dram_tensor("y2f0", [N_CORES * P0_ROWS, FEAT], BF16,
                          kind="Internal", addr_space="Shared"),
           nc.dram_tensor("y2f1", [N_CORES * P1_ROWS, FEAT], BF16,
                          kind="Internal", addr_space="Shared")]

    def call_q(g, h):
        return (2 * g + h) % 4

    with tile.TileContext(nc) as tc:
        with tc.tile_pool(name="const", bufs=1) as cpool, \
             tc.tile_pool(name="slab1", bufs=2) as g1pool, \
             tc.tile_pool(name="slab2", bufs=7) as g2pool, \
             tc.tile_pool(name="oh1p", bufs=2) as oh1pool, \
             tc.tile_pool(name="oh2p", bufs=2) as oh2pool, \
             tc.tile_pool(name="yt", bufs=5) as ypool, \
             tc.tile_pool(name="ht", bufs=3) as hpool, \
             tc.tile_pool(name="ps_a", bufs=2, space="PSUM") as ps_a, \
             tc.tile_pool(name="ps_y", bufs=2, space="PSUM") as ps_y, \
             tc.tile_pool(name="ps_t", bufs=2, space="PSUM") as ps_t:

            def load_const(dram, shape, tag, dtype=F32):
                t = cpool.tile(shape, dtype, tag=tag)
                nc.sync.dma_start(t[:], dram[:])
                return t

            x_sb = load_const(xsb_d, [128, SHARD], "x_sb")
            idx = load_const(idx_d, [128, S2 // 16], "idx", mybir.dt.int16)
            dis = load_const(dis_d, [128, TILES], "dis")
            invdis = load_const(invdis_d, [1, SHARD], "invdis", BF16)
            W1 = load_const(W1_d, [128, 128], "W1", BF16)
            W2 = load_const(W2_d, [128, 128], "W2", BF16)
            b1 = load_const(b1_d, [1, 128], "b1", BF16)
            b2 = load_const(b2_d, [1, 128], "b2", BF16)
            ident = load_const(ident_d, [128, 128], "ident")

            gsem = [nc.alloc_semaphore(f"gsem{q}") for q in range(4)]

            # one gather call: group g, table half h, into the group slab
            def gather_call(g, h, slab):
                grp = _GROUPS[g]
                gbase = cb2[(grp[0], 0)]
                cstart = cb2[(grp[0], h)]
                ncall = int(sum(nch2[t][h] for t in grp))
                ns = ncall * 128
                nc.gpsimd.dma_gather(
                    slab[:, cstart - gbase:cstart - gbase + ncall, :],
                    y2f[h][:, :],
                    idx[:, cstart * 8:cstart * 8 + ns // 16],
                    ns, ns, FEAT,
                    single_packet=False, queue_num=call_q(g, h))

            slabs = {}

            # ---------------- layer 1 ----------------
            for g, grp in enumerate(_GROUPS):
                nch_g = g_nch1[g]
                base = int(cb1[grp[0]])
                slab = g1pool.tile([128, NCHMAX1, 128], FP8, tag="xg")
                nc.sync.dma_start(slab[:, :nch_g, :],
                                  xg_d[:, base:base + nch_g, :])
                ohs = oh1pool.tile([128, NCHMAX1, 128], FP8, tag="oh1")
                nc.sync.dma_start(ohs[:, :nch_g, :],
                                  oh1_d[:, base:base + nch_g, :])
                for t in grp:
                    nt = int(nch1[t])
                    psu = ps_a.tile([128, 128], F32, tag="acc")
                    for k in range(nt):
                        ch = int(cb1[t]) - base + k
                        nc.tensor.matmul(psu[:], slab[:, ch, :],
                                         ohs[:, ch, :],
                                         start=(k == 0), stop=(k == nt - 1))
                    ut = hpool.tile([128, 128], BF16, tag="ut")
                    nc.vector.tensor_copy(ut[:], psu[:])
                    ps2 = ps_y.tile([128, FEAT], F32, tag="ps2")
                    nc.tensor.matmul(ps2[:], ut[:], W1[:],
                                     start=True, stop=False)
                    nc.tensor.matmul(ps2[:], invdis[:, t * 128:(t + 1) * 128],
                                     b1[:], start=False, stop=True)
                    res = ypool.tile([128, FEAT], F32, tag="res")
                    nc.scalar.activation(res[:], ps2[:],
                                         mybir.ActivationFunctionType.Relu,
                                         scale=dis[:, t:t + 1])
                    nc.vector.tensor_tensor(res[:], res[:],
                                            x_sb[:, t * 128:(t + 1) * 128],
                                            mybir.AluOpType.add)
                    pst = ps_t.tile([128, 128], F32)
                    nc.tensor.transpose(pst[:], res[:], ident[:])
                    hT = hpool.tile([128, 128], BF16, tag="hT")
                    nc.vector.tensor_copy(hT[:], pst[:])
                    ps2b = ps_y.tile([128, FEAT], F32, tag="ps2")
                    nc.tensor.matmul(ps2b[:], hT[:], W2[:],
                                     start=True, stop=True)
                    y2t = ypool.tile([128, FEAT], BF16, tag="y2t")
                    nc.scalar.activation(y2t[:], ps2b[:],
                                         mybir.ActivationFunctionType.Copy,
                                         scale=dis[:, t:t + 1])
                    if t < P0_TILES:
                        nc.sync.dma_start(
                            y2s[0][t * 128:(t + 1) * 128, :], y2t[:])
                    else:
                        tt = t - P0_TILES
                        nc.sync.dma_start(
                            y2s[1][tt * 128:(tt + 1) * 128, :], y2t[:])
                if grp[-1] == P0_TILES - 1:
                    nc.gpsimd.collective_compute(
                        "AllGather", mybir.AluOpType.bypass,
                        replica_groups=[list(range(N_CORES))],
                        ins=[y2s[0][:, :]], outs=[y2f[0][:, :]])
            # early h0 gathers: need only table half 0 (collective0), so
            # their descriptor generation overlaps the L1 tail + collective1
            for g in (0, 1, 2, 3, 4, 5):
                slabs[g] = g2pool.tile([128, NCHMAX2, 128], BF16, tag="slab", name="slab_e")
                gather_call(g, 0, slabs[g])
            nc.gpsimd.collective_compute(
                "AllGather", mybir.AluOpType.bypass,
                replica_groups=[list(range(N_CORES))],
                ins=[y2s[1][:, :]], outs=[y2f[1][:, :]])

            # ---------------- layer 2 ----------------
            for g, grp in enumerate(_GROUPS):
                if g in slabs:
                    slab = slabs.pop(g)
                    gather_call(g, 1, slab)
                else:
                    slab = g2pool.tile([128, NCHMAX2, 128], BF16, tag="slab")
                    gather_call(g, 0, slab)
                    gather_call(g, 1, slab)
                gbase = cb2[(grp[0], 0)]
                ohs = oh2pool.tile([128, NCHMAX2, 128], BF16, tag="oh2")
                nc.sync.dma_start(ohs[:, :g_nch2[g], :],
                                  oh2_d[:, gbase:gbase + g_nch2[g], :])
                for t in grp:
                    yo = ypool.tile([128, FEAT], BF16, tag="yo")
                    if t < P0_TILES:
                        nc.sync.dma_start(
                            yo[:], y2s[0][t * 128:(t + 1) * 128, :])
                    else:
                        tt = t - P0_TILES
                        nc.sync.dma_start(
                            yo[:], y2s[1][tt * 128:(tt + 1) * 128, :])
                    ps = ps_a.tile([128, FEAT], F32, tag="acc")
                    nc.tensor.matmul(ps[:], invdis[:, t * 128:(t + 1) * 128],
                                     b2[:], start=True, stop=False)
                    pairs = [(h, k) for h in (0, 1)
                             for k in range(int(nch2[t][h]))]
                    for j, (h, k) in enumerate(pairs):
                        ch = cb2[(t, h)] - gbase + k
                        nc.tensor.matmul(ps[:], ohs[:, ch, :], slab[:, ch, :],
                                         start=False,
                                         stop=(j == len(pairs) - 1))
                    # self-loop: P += y2own (y2 rows already carry dis_src)
                    nc.vector.tensor_tensor(ps[:], ps[:], yo[:],
                                            mybir.AluOpType.add)
                    res = ypool.tile([128, FEAT], F32, tag="res")
                    nc.scalar.activation(res[:], ps[:],
                                         mybir.ActivationFunctionType.Copy,
                                         scale=dis[:, t:t + 1])
                    nc.sync.dma_start(out_d[t * 128:(t + 1) * 128, :], res[:])

    if compile:
        nc.compile()
    return nc


_CACHE = {}


def kernel(edge_index, x, W1, b1, W2, b2, _trace=False):
    per_core, spec, nch1, nch2, cb1, cb2, S1, S2, NCH2 = _preprocess(
        edge_index, x, W1, b1, W2, b2)

    key = hashlib.sha1(repr(spec).encode()).hexdigest()
    if key not in _CACHE:
        _CACHE[key] = _build(nch1, nch2, cb1, cb2, S2, NCH2)
    nc = _CACHE[key]

    res = run_bass_kernel_spmd(nc, per_core, core_ids=list(range(N_CORES)),
                               trace=_trace)
    out = np.concatenate([res.results[c]["out"] for c in range(N_CORES)],
                         axis=0)[:N_REAL]
    if _trace:
        return out, res
    return out
